# revision 13
# baseline (speedup 1.0000x reference)
"""ConMamba-CTC Trainium2 kernel: 8 NeuronCores, batch(4) x time(2) sharding.

Each core handles (batch b, time-half h): frontend convs + 8 Mamba blocks +
CTC head for its 256 mamba timesteps. Cross-core coupling per block:
  AG_h: scan carry h_mid (phase-1 scan end state, h0=0)
  AG_x: 3-row x halo for the next block's causal dwconv / in_proj
Both are pairwise AllGathers; odd-half cores consume partner data (masked by
the hmask input), even-half cores multiply by 0 (causal start).
The scan runs twice: phase 1 (gpsimd) produces the carry; phase 2 (DVE)
re-scans with the carry injected via a reset column (dA=0, dBx=h_init), which
chains the recurrence across the flattened (n, 257) free layout.
"""
import numpy as np
import ml_dtypes

import concourse.bass as bass
import concourse.tile as tile
from concourse import mybir
from concourse.bass_utils import run_bass_kernel_spmd

F32 = mybir.dt.float32
BF16 = mybir.dt.bfloat16
BF = ml_dtypes.bfloat16
AF = mybir.ActivationFunctionType
OP = mybir.AluOpType

D = 512
DI = 1024
NB = 8
NS = 16
DTR = 32
CK = 4
MEL = 80
VOCAB = 2048
B, TFULL = 4, 2048
TL = 256
TT = TL + 1
FEATC = 1028
C1W = 513
KG = D // 128     # 4
GI = DI // 128    # 8
P = 128

SCAN1_ENGINE = "vector"   # gpsimd scan rejected by walrus (Pool engine check)
DA_DTYPE = BF16           # scan decay dtype (flip to F32 if rel-err too high)


def _split_multi_waits(nc):
    """This walrus build accepts one sync-wait per ISA instruction; hoist
    extras onto single-wait NoOps on the same engine."""
    cnt = 0
    for f in nc.m.functions:
        for bb in f.blocks:
            out, changed = [], False
            for inst in bb.instructions:
                si = inst.sync_info
                waits = list(si.on_wait) if si and si.on_wait else []
                if len(waits) > 1:
                    for w in waits[:-1]:
                        nop = mybir.InstNoOp(name=f"waitnop_{cnt}", ins=[], outs=[])
                        cnt += 1
                        nop.engine = inst.engine
                        nop.sync_info = mybir.SyncInfo(on_wait=[w], on_update=[])
                        out.append(nop)
                    inst.sync_info = mybir.SyncInfo(
                        on_wait=[waits[-1]],
                        on_update=list(si.on_update) if si.on_update else [])
                    changed = True
                out.append(inst)
            if changed:
                bb.instructions = out
    return cnt


def _nbcast_g(t, g, n):
    """Read tile t[:, g, :] ([128, G, T]) broadcast n times along a new
    leading free dim (step 0) -> logical [128, n, T]."""
    sl = t[:, g, :]
    return bass.AP(tensor=sl.tensor, offset=sl.offset,
                   ap=[sl.ap[0], [0, n], sl.ap[1]])


def _fbcast(sl, count):
    """Per-partition scalar [128, 1] broadcast along free dim."""
    return bass.AP(tensor=sl.tensor, offset=sl.offset, ap=[sl.ap[0], [0, count]])


def build_nc():
    nc = bass.Bass(num_devices=8)

    def ein(name, shape, dt=F32):
        return nc.dram_tensor(name, shape, dt, kind="ExternalInput")

    feats = ein("featsT", [MEL, FEATC], BF16)
    w1T = ein("w1T", [MEL, 3, D], BF16)
    b1 = ein("b1", [P, KG])
    w2T = ein("w2T", [P, KG, 3, D], BF16)
    b2 = ein("b2", [P, KG])
    w1p = ein("w1p", [NB, P, KG, 2 * DI], BF16)
    xpw = ein("xpw", [NB, P, GI, DTR + 2 * NS], BF16)
    dpw = ein("dpw", [NB, DTR, DI], BF16)
    dpb = ein("dpb", [NB, P, GI])
    cw = ein("cw", [NB, P, GI, CK])
    cb = ein("cb", [NB, P, GI])
    aneg = ein("aneg", [NB, P, GI, NS])
    dsk = ein("dsk", [NB, P, GI])
    opw = ein("opw", [NB, P, GI, D], BF16)
    hw = ein("hw", [P, KG, VOCAB], BF16)
    hmask = ein("hmask", [P, 1])

    logits = nc.dram_tensor("logits", [TL, VOCAB], F32, kind="ExternalOutput")

    # carry buffers: h AGs (one per block) and x-halo AGs (pre + per block)
    HSZ = GI * P * NS
    XSZ = KG * P * 3
    hco = [nc.dram_tensor(f"hco{k}", [HSZ], BF16) for k in range(NB)]
    hci = [nc.dram_tensor(f"hci{k}", [2 * HSZ], BF16) for k in range(NB)]
    xco = [nc.dram_tensor(f"xco{k}", [XSZ], F32) for k in range(NB)]
    xci = [nc.dram_tensor(f"xci{k}", [2 * XSZ], F32) for k in range(NB)]
    rg = [[0, 1], [2, 3], [4, 5], [6, 7]]

    with tile.TileContext(nc) as tc:
        import contextlib
        ctx = contextlib.ExitStack()
        with ctx:
            sing = ctx.enter_context(tc.tile_pool(name="sing", bufs=1))
            front = ctx.enter_context(tc.tile_pool(name="front", bufs=1))
            wpool = ctx.enter_context(tc.tile_pool(name="wpool", bufs=1))
            apool = ctx.enter_context(tc.tile_pool(name="apool", bufs=1))
            spool = ctx.enter_context(tc.tile_pool(name="spool", bufs=2))
            small = ctx.enter_context(tc.tile_pool(name="small", bufs=2))
            ppm = ctx.enter_context(tc.tile_pool(name="ppm", bufs=3, space="PSUM"))
            ppstat = ctx.enter_context(tc.tile_pool(name="ppstat", bufs=2, space="PSUM"))
            ppx = ctx.enter_context(tc.tile_pool(name="ppx", bufs=1, space="PSUM"))

            ones = sing.tile([1, P], F32)
            nc.vector.memset(ones, 1.0)
            onc = sing.tile([P, 1], BF16)
            nc.vector.memset(onc, 1.0)
            hm = sing.tile([P, 1], F32)
            nc.sync.dma_start(out=hm, in_=hmask[:, :])

            x = sing.tile([P, KG, TL], F32)
            xin = sing.tile([P, KG, 3 + TL], BF16)

            # ---------------- frontend ----------------
            fp_ctx = tc.tile_pool(name="fpool", bufs=1)
            fpool = fp_ctx.__enter__()
            fsb = fpool.tile([MEL, FEATC], BF16)
            nc.sync.dma_start(out=fsb, in_=feats[:, :])
            w1s = fpool.tile([MEL, 3, D], BF16)
            nc.sync.dma_start(out=w1s, in_=w1T[:, :, :])
            b1s = fpool.tile([P, KG], F32, tag="b1s")
            nc.sync.dma_start(out=b1s, in_=b1[:, :])
            c1 = fpool.tile([P, KG, C1W], BF16)
            for og in range(KG):
                for c0, cn in ((0, 257), (257, 256)):
                    ps = ppm.tile([P, 512], F32, tag="mm")
                    for kk in range(3):
                        rhs = bass.AP(tensor=fsb.tensor,
                                      offset=fsb.offset + kk + 2 * c0,
                                      ap=[fsb.ap[0], [2, cn]])
                        nc.tensor.matmul(ps[:, :cn],
                                         lhsT=w1s[:, kk, og * P:(og + 1) * P],
                                         rhs=rhs, start=(kk == 0), stop=(kk == 2))
                    nc.scalar.activation(c1[:, og, c0:c0 + cn], ps[:, :cn],
                                         AF.Gelu, bias=b1s[:, og:og + 1])
            w2s = fpool.tile([P, KG, 3, D], BF16)
            nc.sync.dma_start(out=w2s, in_=w2T[:, :, :, :])
            b2s = fpool.tile([P, KG], F32, tag="b2s")
            nc.sync.dma_start(out=b2s, in_=b2[:, :])
            for og in range(KG):
                ps = ppm.tile([P, 512], F32, tag="mm")
                first = True
                for ig in range(KG):
                    for kk in range(3):
                        rhs = bass.AP(tensor=c1.tensor,
                                      offset=c1.offset + ig * C1W + kk,
                                      ap=[c1.ap[0], [2, TL]])
                        nc.tensor.matmul(ps[:, :TL],
                                         lhsT=w2s[:, ig, kk, og * P:(og + 1) * P],
                                         rhs=rhs, start=first,
                                         stop=(ig == KG - 1 and kk == 2))
                        first = False
                nc.scalar.activation(x[:, og, :], ps[:, :TL], AF.Gelu,
                                     bias=b2s[:, og:og + 1])
                nc.vector.tensor_copy(xin[:, og, 3:], x[:, og, :])

            fp_ctx.__exit__(None, None, None)

            # pre-loop x-halo AG (block 0 halo)
            for g in range(KG):
                nc.sync.dma_start(
                    out=xco[0][g * 384:(g + 1) * 384].rearrange("(p c) -> p c", p=P),
                    in_=x[:, g, TL - 3:TL])
            nc.gpsimd.collective_compute("AllGather", OP.bypass, replica_groups=rg,
                                         ins=[xco[0].ap()], outs=[xci[0].ap()])

            # ---------------- blocks ----------------
            for blk in range(NB):
                xh = small.tile([P, KG, 3], F32, tag="xh")
                nc.sync.dma_start(
                    out=xh, in_=xci[blk][0:XSZ].rearrange("(g p c) -> p g c", p=P, g=KG))
                nc.vector.tensor_scalar(xin[:, :, 0:3], xh, hm, None, op0=OP.mult)

                w1t = wpool.tile([P, KG, 2 * DI], BF16, tag="w1p")
                nc.sync.dma_start(out=w1t, in_=w1p[blk])
                xpt = wpool.tile([P, GI, DTR + 2 * NS], BF16, tag="xpw")
                nc.sync.dma_start(out=xpt, in_=xpw[blk])
                dpt = wpool.tile([DTR, DI], BF16, tag="dpw")
                nc.sync.dma_start(out=dpt, in_=dpw[blk])
                dpbt = wpool.tile([P, GI], F32, tag="dpb")
                nc.sync.dma_start(out=dpbt, in_=dpb[blk])
                cwt = wpool.tile([P, GI, CK], F32, tag="cw")
                nc.sync.dma_start(out=cwt, in_=cw[blk])
                cbt = wpool.tile([P, GI], F32, tag="cb")
                nc.sync.dma_start(out=cbt, in_=cb[blk])
                ant = wpool.tile([P, GI, NS], F32, tag="aneg")
                nc.sync.dma_start(out=ant, in_=aneg[blk])
                dskt = wpool.tile([P, GI], F32, tag="dsk")
                nc.sync.dma_start(out=dskt, in_=dsk[blk])
                opt = wpool.tile([P, GI, D], BF16, tag="opw")
                nc.sync.dma_start(out=opt, in_=opw[blk])

                # --- RMSNorm scale r (main + halo separately, halo waits AG_x) ---
                xsq = apool.tile([P, KG, 3 + TL], BF16, tag="xsq")
                nc.vector.tensor_mul(xsq[:, :, 3:], xin[:, :, 3:], xin[:, :, 3:])
                nc.vector.tensor_mul(xsq[:, :, 0:3], xin[:, :, 0:3], xin[:, :, 0:3])
                psr = ppstat.tile([1, 512], F32, tag="stat")
                for g in range(KG):
                    nc.tensor.matmul(psr[:, 0:TL], lhsT=onc, rhs=xsq[:, g, 3:],
                                     start=(g == 0), stop=(g == KG - 1))
                psrh = ppstat.tile([1, 512], F32, tag="stat")
                for g in range(KG):
                    nc.tensor.matmul(psrh[:, 0:3], lhsT=onc, rhs=xsq[:, g, 0:3],
                                     start=(g == 0), stop=(g == KG - 1))
                mt = small.tile([1, 3 + TL], F32, tag="mt")
                nc.vector.tensor_scalar(mt[:, 3:], psr[:, 0:TL], 1.0 / D, 1e-5,
                                        op0=OP.mult, op1=OP.add)
                nc.vector.tensor_scalar(mt[:, 0:3], psrh[:, 0:3], 1.0 / D, 1e-5,
                                        op0=OP.mult, op1=OP.add)
                qt = small.tile([1, 3 + TL], F32, tag="qt")
                nc.scalar.activation(qt, mt, AF.Ln)
                rr = small.tile([1, 3 + TL], F32, tag="rr")
                nc.scalar.activation(rr, qt, AF.Exp, scale=-0.5)
                psrr = ppm.tile([P, 512], F32, tag="mm")
                nc.tensor.matmul(psrr[:, :3 + TL], lhsT=ones, rhs=rr,
                                 start=True, stop=True)
                rrep = apool.tile([P, 3 + TL], F32, tag="rrep")
                nc.scalar.copy(rrep, psrr[:, :3 + TL])

                # --- in_proj: main (cols 3:259) then halo (cols 0:3) ---
                xs = apool.tile([P, GI, 3 + TL], BF16, tag="xs")
                zb = apool.tile([P, GI, TL], BF16, tag="zb")
                for cg in range(2 * GI):
                    ps = ppm.tile([P, 512], F32, tag="mm")
                    for kg in range(KG):
                        nc.tensor.matmul(ps[:, :TL],
                                         lhsT=w1t[:, kg, cg * P:(cg + 1) * P],
                                         rhs=xin[:, kg, 3:], start=(kg == 0),
                                         stop=(kg == KG - 1))
                    if cg < GI:
                        nc.vector.tensor_mul(xs[:, cg, 3:], ps[:, :TL], rrep[:, 3:])
                    else:
                        nc.vector.tensor_mul(zb[:, cg - GI, :], ps[:, :TL], rrep[:, 3:])
                for cg in range(GI):   # halo cols for xs only
                    ps = ppm.tile([P, 512], F32, tag="mm")
                    for kg in range(KG):
                        nc.tensor.matmul(ps[:, 0:3],
                                         lhsT=w1t[:, kg, cg * P:(cg + 1) * P],
                                         rhs=xin[:, kg, 0:3], start=(kg == 0),
                                         stop=(kg == KG - 1))
                    nc.vector.tensor_mul(xs[:, cg, 0:3], ps[:, 0:3], rrep[:, 0:3])

                # --- causal dwconv + silu ---
                t1 = apool.tile([P, GI, TL], BF16, tag="t1")
                zg = apool.tile([P, GI, TL], BF16, tag="zg")
                for g in range(GI):
                    acc = small.tile([P, 2, TL], F32, tag="convacc")
                    nc.vector.scalar_tensor_tensor(
                        acc[:, 0, :], xs[:, g, 0:TL], cwt[:, g, 0:1],
                        _fbcast(cbt[:, g:g + 1], TL), op0=OP.mult, op1=OP.add)
                    nc.vector.scalar_tensor_tensor(
                        acc[:, 1, :], xs[:, g, 1:1 + TL], cwt[:, g, 1:2],
                        acc[:, 0, :], op0=OP.mult, op1=OP.add)
                    nc.vector.scalar_tensor_tensor(
                        acc[:, 0, :], xs[:, g, 2:2 + TL], cwt[:, g, 2:3],
                        acc[:, 1, :], op0=OP.mult, op1=OP.add)
                    nc.vector.scalar_tensor_tensor(
                        acc[:, 1, :], xs[:, g, 3:3 + TL], cwt[:, g, 3:4],
                        acc[:, 0, :], op0=OP.mult, op1=OP.add)
                    nc.scalar.activation(t1[:, g, :], acc[:, 1, :], AF.Silu)
                    nc.scalar.activation(zg[:, g, :], zb[:, g, :], AF.Silu)

                # --- x_proj ---
                psx = ppx.tile([64, 512], F32, tag="psx")
                for g in range(GI):
                    nc.tensor.matmul(psx[:, :TL], lhsT=xpt[:, g, :], rhs=t1[:, g, :],
                                     start=(g == 0), stop=(g == GI - 1))
                dts = small.tile([DTR, TL], BF16, tag="dts")
                nc.scalar.copy(dts, psx[0:DTR, :TL])
                bcs = small.tile([2 * NS, TL], F32, tag="bcs")
                nc.scalar.copy(bcs, psx[DTR:DTR + 2 * NS, :TL])
                brep = apool.tile([P, NS, TL], BF16, tag="brep")
                crep = apool.tile([P, NS, TL], BF16, tag="crep")
                for half, dst in ((0, brep), (1, crep)):
                    for c in range(NS * TL // 512):
                        bcf = small.tile([1, 512], F32, tag="bcf")
                        nc.sync.dma_start(
                            out=bcf.rearrange("o (n t) -> o n t", n=2),
                            in_=bcs[half * NS + 2 * c:half * NS + 2 * c + 2, :])
                        psb = ppm.tile([P, 512], F32, tag="mm")
                        nc.tensor.matmul(psb, lhsT=ones, rhs=bcf,
                                         start=True, stop=True)
                        nc.scalar.copy(
                            dst.rearrange("p n t -> p (n t)")[:, c * 512:(c + 1) * 512],
                            psb)

                # --- dt_proj + softplus; u = delta * t1 ---
                dlt = apool.tile([P, GI, TL], F32, tag="dlt")
                u = apool.tile([P, GI, TL], BF16, tag="u")
                for g in range(GI):
                    psd = ppm.tile([P, 512], F32, tag="mm")
                    nc.tensor.matmul(psd[:, :TL], lhsT=dpt[:, g * P:(g + 1) * P],
                                     rhs=dts, start=True, stop=True)
                    spt = small.tile([P, TL], F32, tag="spt")
                    nc.scalar.activation(spt, psd[:, :TL], AF.Exp,
                                         bias=dpbt[:, g:g + 1])
                    nc.scalar.activation(dlt[:, g, :], spt, AF.Ln, bias=1.0)
                    nc.vector.tensor_mul(u[:, g, :], dlt[:, g, :], t1[:, g, :])

                scan1 = nc.gpsimd if SCAN1_ENGINE == "gpsimd" else nc.vector

                def build_da_dbx(g):
                    da = spool.tile([P, NS, TT], DA_DTYPE, tag="da")
                    dbx = spool.tile([P, NS, TT], BF16, tag="dbx")
                    nc.vector.memset(da[:, :, 0:1], 0.0)
                    for n in range(NS):
                        nc.scalar.activation(da[:, n, 1:], dlt[:, g, :], AF.Exp,
                                             scale=ant[:, g, n:n + 1])
                    nc.vector.tensor_tensor(dbx[:, :, 1:], _nbcast_g(u, g, NS),
                                            brep, op=OP.mult)
                    return da, dbx

                # --- scan phase 1 (h0 = 0) -> h_mid carry ---
                for g in range(GI):
                    da, dbx = build_da_dbx(g)
                    nc.vector.memset(dbx[:, :, 0], 0.0)
                    hh = spool.tile([P, NS, TT], BF16, tag="hh")
                    scan1.tensor_tensor_scan(
                        hh.rearrange("p n t -> p (n t)"),
                        da.rearrange("p n t -> p (n t)"),
                        dbx.rearrange("p n t -> p (n t)"),
                        0.0, op0=OP.mult, op1=OP.add)
                    nc.scalar.dma_start(
                        out=hco[blk][g * P * NS:(g + 1) * P * NS].rearrange(
                            "(p n) -> p n", p=P),
                        in_=hh[:, :, TT - 1])
                nc.gpsimd.collective_compute("AllGather", OP.bypass,
                                             replica_groups=rg,
                                             ins=[hco[blk].ap()], outs=[hci[blk].ap()])
                hca = small.tile([P, GI, NS], BF16, tag="hca")
                nc.sync.dma_start(
                    out=hca, in_=hci[blk][0:HSZ].rearrange("(g p n) -> p g n", p=P, g=GI))
                hin = small.tile([P, GI, NS], F32, tag="hin")
                nc.vector.tensor_scalar(hin, hca, hm, None, op0=OP.mult)

                # --- scan phase 2 (carry injected) + y tail ---
                y3 = apool.tile([P, GI, TL], BF16, tag="y3")
                for g in range(GI):
                    da, dbx = build_da_dbx(g)
                    nc.vector.tensor_copy(dbx[:, :, 0], hin[:, g, :])
                    hh = spool.tile([P, NS, TT], BF16, tag="hh")
                    nc.vector.tensor_tensor_scan(
                        hh.rearrange("p n t -> p (n t)"),
                        da.rearrange("p n t -> p (n t)"),
                        dbx.rearrange("p n t -> p (n t)"),
                        0.0, op0=OP.mult, op1=OP.add)
                    # y = sum_n C*h (ping-pong tree through dbx/da)
                    nc.vector.tensor_tensor(dbx[:, :, 1:], hh[:, :, 1:], crep,
                                            op=OP.mult)
                    nc.vector.tensor_add(da[:, 0:8, 1:], dbx[:, 0:8, 1:],
                                         dbx[:, 8:16, 1:])
                    nc.vector.tensor_add(dbx[:, 0:4, 1:], da[:, 0:4, 1:],
                                         da[:, 4:8, 1:])
                    nc.vector.tensor_add(da[:, 0:2, 1:], dbx[:, 0:2, 1:],
                                         dbx[:, 2:4, 1:])
                    nc.vector.tensor_add(dbx[:, 0, 1:], da[:, 0, 1:], da[:, 1, 1:])
                    nc.vector.scalar_tensor_tensor(
                        da[:, 0, 1:], t1[:, g, :], dskt[:, g:g + 1], dbx[:, 0, 1:],
                        op0=OP.mult, op1=OP.add)
                    nc.vector.tensor_mul(y3[:, g, :], da[:, 0, 1:], zg[:, g, :])

                # --- out_proj + residual ---
                for mg in range(KG):
                    pso = ppm.tile([P, 512], F32, tag="mm")
                    for g in range(GI):
                        nc.tensor.matmul(pso[:, :TL],
                                         lhsT=opt[:, g, mg * P:(mg + 1) * P],
                                         rhs=y3[:, g, :], start=(g == 0),
                                         stop=(g == GI - 1))
                    nc.vector.tensor_add(x[:, mg, :], x[:, mg, :], pso[:, :TL])
                    nc.vector.tensor_copy(xin[:, mg, 3:], x[:, mg, :])

                # --- x-halo AG for next block ---
                if blk + 1 < NB:
                    for g in range(KG):
                        nc.sync.dma_start(
                            out=xco[blk + 1][g * 384:(g + 1) * 384].rearrange(
                                "(p c) -> p c", p=P),
                            in_=x[:, g, TL - 3:TL])
                    nc.gpsimd.collective_compute(
                        "AllGather", OP.bypass, replica_groups=rg,
                        ins=[xco[blk + 1].ap()], outs=[xci[blk + 1].ap()])

            # ---------------- head ----------------
            hws = front.tile([P, KG, VOCAB], BF16, tag="hw")
            nc.sync.dma_start(out=hws, in_=hw[:, :, :])
            for tg in range(TL // P):
                for vc in range(VOCAB // 512):
                    psh = ppm.tile([P, 512], F32, tag="mm")
                    for kg in range(KG):
                        nc.tensor.matmul(psh,
                                         lhsT=xin[:, kg, 3 + tg * P:3 + (tg + 1) * P],
                                         rhs=hws[:, kg, vc * 512:(vc + 1) * 512],
                                         start=(kg == 0), stop=(kg == KG - 1))
                    lg = front.tile([P, 512], F32, tag="lg")
                    nc.scalar.copy(lg, psh)
                    nc.sync.dma_start(out=logits[tg * P:(tg + 1) * P,
                                                 vc * 512:(vc + 1) * 512], in_=lg)

    _split_multi_waits(nc)
    return nc


_NC_CACHE = None


def _get_nc():
    global _NC_CACHE
    if _NC_CACHE is None:
        _NC_CACHE = build_nc()
    return _NC_CACHE


def _prep_host(inputs):
    f32 = np.float32
    g = lambda k: np.asarray(inputs[k], f32)
    feats, conv1_w, conv1_b = g("feats"), g("conv1_w"), g("conv1_b")
    conv2_w, conv2_b = g("conv2_w"), g("conv2_b")
    norm_w, in_proj_w = g("norm_w"), g("in_proj_w")
    dwconv_w, dwconv_b = g("dwconv_w"), g("dwconv_b")
    x_proj_w, dt_proj_w, dt_proj_b = g("x_proj_w"), g("dt_proj_w"), g("dt_proj_b")
    A_log, Dskip, out_proj_w = g("A_log"), g("Dskip"), g("out_proj_w")
    head_w, head_b = g("head_w"), g("head_b")

    def grpv(a):  # [NB, DI] -> [NB, 128, GI]
        return np.ascontiguousarray(a.reshape(NB, GI, P).transpose(0, 2, 1))

    shared = {
        "w1T": np.ascontiguousarray(conv1_w.transpose(1, 2, 0)).astype(BF),
        "b1": np.ascontiguousarray(conv1_b.reshape(KG, P).T.astype(f32)),
        "w2T": np.ascontiguousarray(
            conv2_w.transpose(1, 2, 0).reshape(KG, P, 3, D)
            .transpose(1, 0, 2, 3)).astype(BF),
        "b2": np.ascontiguousarray(conv2_b.reshape(KG, P).T.astype(f32)),
        "w1p": np.ascontiguousarray(
            (norm_w[:, :, None] * in_proj_w).reshape(NB, KG, P, 2 * DI)
            .transpose(0, 2, 1, 3)).astype(BF),
        "xpw": np.ascontiguousarray(
            x_proj_w.reshape(NB, GI, P, DTR + 2 * NS).transpose(0, 2, 1, 3)).astype(BF),
        "dpw": dt_proj_w.astype(BF),
        "dpb": grpv(dt_proj_b),
        "cw": np.ascontiguousarray(dwconv_w.reshape(NB, GI, P, CK).transpose(0, 2, 1, 3)),
        "cb": grpv(dwconv_b),
        "aneg": np.ascontiguousarray(
            (-np.exp(A_log)).reshape(NB, GI, P, NS).transpose(0, 2, 1, 3)),
        "dsk": grpv(Dskip),
        "opw": np.ascontiguousarray(
            out_proj_w.reshape(NB, GI, P, D).transpose(0, 2, 1, 3)).astype(BF),
        "hw": np.ascontiguousarray(head_w.reshape(KG, P, VOCAB).transpose(1, 0, 2)).astype(BF),
    }
    in_maps = []
    for core in range(8):
        b, h = core // 2, core % 2
        lo = h * 1024 - 3
        fc = np.zeros((FEATC, MEL), f32)
        s, e = max(0, lo), min(TFULL, lo + FEATC)
        fc[s - lo:e - lo] = feats[b, s:e, :]
        m = dict(shared)
        m["featsT"] = np.ascontiguousarray(fc.T).astype(BF)
        m["hmask"] = np.full((P, 1), float(h), f32)
        in_maps.append(m)
    return in_maps


LAST_RESULT = None


def kernel(**inputs):
    global LAST_RESULT
    import os
    nc = _get_nc()
    in_maps = _prep_host(inputs)
    trace = bool(int(os.environ.get("K_TRACE", "0")))
    res = run_bass_kernel_spmd(nc, in_maps, core_ids=list(range(8)), trace=trace)
    LAST_RESULT = res
    logits = np.zeros((B, 2 * TL, VOCAB), np.float32)
    for core in range(8):
        b, h = core // 2, core % 2
        logits[b, h * TL:(h + 1) * TL, :] = res.results[core]["logits"]
    logits += np.asarray(inputs["head_b"], np.float32)[None, None, :]
    feat_lens = np.asarray(inputs["feat_lens"]).astype(np.int64)
    out_lens = np.maximum(feat_lens // 4, 1).astype(np.int32)
    return logits, out_lens
